# revision 15
# baseline (speedup 1.0000x reference)
"""ConvModLayer (StyleGAN2-style modulated 3x3 conv) on 8 Trainium2
NeuronCores — data-parallel over the batch (16 samples -> 2 per core),
computed via Winograd F(2x2, 3x3) in bf16.

Host folds modulation/demodulation into per-sample weights, applies the
Winograd filter transform G w G^T, AND the data transform B^T d B (so
the device receives ready-to-matmul V tiles in bf16):
  weff[b,o,i]   = w * c * s[b,i] * rsqrt(sigma_sq[b,o]+eps)
  Wt[b,uv,o,i]  = (G weff G^T)[uv]
  V[b,uv,i,t]   = (B^T d B)[uv]   per 4x4 input tile (stride 2)
Device per quarter-image (8 tile-rows x 32 tile-cols = 256 tiles of
2x2 outputs) and output-channel chunk:
  M[uv] = Wt[uv] @ V[uv]      -- 64 matmuls (free 256), PSUM f32
  Y = A^T M A                 -- batched DVE adds (ACT helps drain PSUM)
16*512*512*1024 MACs per sample vs 9*512*512*4096 direct: 2.25x fewer
PE cycles (2048 matmuls of 256 rows vs 2304 of 512).
"""

import sys
from contextlib import ExitStack

if "/opt/trn_rl_repo" not in sys.path:
    sys.path.insert(0, "/opt/trn_rl_repo")

import ml_dtypes
import numpy as np

import concourse.bacc as bacc
import concourse.mybir as mybir
import concourse.tile as tile
from concourse.bass_utils import run_bass_kernel_spmd

F32 = mybir.dt.float32
BF16 = mybir.dt.bfloat16
BF16_NP = np.dtype(ml_dtypes.bfloat16)

N_CORES = 8
B = 16
B2 = B // N_CORES  # samples per core
C = 512
NCH = 4  # 128-partition channel chunks
H = W = 64
HP = H + 2  # zero-padded
EPS = 1e-8
CSCALE = 1.0 / (C * 9) ** 0.5

_NC_CACHE = {}


def _build():
    nc = bacc.Bacc("TRN2", target_bir_lowering=False, debug=False)

    v_d = nc.dram_tensor("v", [B2, 4, 128, 64, 8, 32], BF16, kind="ExternalInput")
    w_d = nc.dram_tensor("w", [B2, NCH, 128, 16, NCH, 128], BF16, kind="ExternalInput")
    o_d = nc.dram_tensor("o", [B2, C, H, W], F32, kind="ExternalOutput")

    with tile.TileContext(nc) as tc, ExitStack() as ctx:
        vpool = ctx.enter_context(tc.tile_pool(name="vpool", bufs=2))
        wpool = ctx.enter_context(tc.tile_pool(name="wpool", bufs=5))
        zpool = ctx.enter_context(tc.tile_pool(name="zpool", bufs=8))
        ztpool = ctx.enter_context(tc.tile_pool(name="ztpool", bufs=3))
        ypool = ctx.enter_context(tc.tile_pool(name="ypool", bufs=3))
        pspool = ctx.enter_context(
            tc.tile_pool(name="pspool", bufs=8, space="PSUM")
        )

        quarters = [(b, q) for b in range(B2) for q in range(4)]

        def emit_v_dma(b, q):
            vt = vpool.tile([128, 64, 8, 32], BF16, tag="v", name=f"v{b}{q}")
            nc.sync.dma_start(vt[:], v_d[b, q])
            return vt

        def emit_w_dma(b, occ):
            wt = wpool.tile([128, 16, NCH, 128], BF16, tag="w", name=f"w{b}{occ}")
            nc.sync.dma_start(wt[:], w_d[b, occ])
            return wt

        # warm-up, in first-group consumption order: the (u,vi,ic) matmul
        # loop eats V dim1 in 8-slot blocks {0,16,32,48,8,24,40,56} and w
        # dim1 in 2-slot blocks {0,4,8,12,2,6,10,14}, so chunk the first
        # quarter's DMAs that way — first matmul then waits on ~6KB per
        # partition instead of the full 48KB.
        wts = {}
        v0 = vpool.tile([128, 64, 8, 32], BF16, tag="v", name="v00")
        w0 = wpool.tile([128, 16, NCH, 128], BF16, tag="w", name="w00")
        for vh in range(2):
            for u in range(4):
                j = u * 16 + vh * 8
                k = u * 4 + vh * 2
                nc.sync.dma_start(w0[:, k : k + 2], w_d[0, 0, :, k : k + 2])
                nc.sync.dma_start(v0[:, j : j + 8], v_d[0, 0, :, j : j + 8])
        vqs = {0: v0}
        wts[(0, 0)] = w0
        for occ in range(1, NCH):
            wts[(0, occ)] = emit_w_dma(0, occ)
        vqs[1] = emit_v_dma(0, 1)

        for qi, (b, q) in enumerate(quarters):
            vt = vqs[qi]
            for occ in range(NCH):
                wt = wts[(b, occ)]
                # zt dim1 = zu*4 + v
                zt = ztpool.tile([128, 8, 8, 32], BF16, tag="zt", name="zt")
                for vh in range(2):
                    ms = [
                        pspool.tile([128, 2, 8, 32], F32, tag="ps", name=f"m{u}")
                        for u in range(4)
                    ]
                    for u in range(4):
                        for vi in range(2):
                            uv = u * 4 + 2 * vh + vi
                            for ic in range(NCH):
                                nc.tensor.matmul(
                                    ms[u][:, vi],
                                    wt[:, uv, ic, :],
                                    vt[:, uv * 4 + ic],
                                    start=(ic == 0 and vi == 0),
                                    stop=(ic == 3 and vi == 1),
                                    skip_group_check=True,
                                )
                    # Z row stage (A^T M), batched over both v of this half:
                    #   Z0 = M0+M1+M2 ; Z1 = M1-M2-M3
                    # TensorTensor may read only ONE PSUM input, so M1 goes
                    # through an ACT copy; each DVE op reads one PSUM operand.
                    c1 = zpool.tile([128, 2, 8, 32], BF16, tag="zp", name="c1")
                    nc.scalar.copy(c1[:], ms[1][:])
                    t0 = zpool.tile([128, 2, 8, 32], BF16, tag="zp", name="t0")
                    nc.vector.tensor_add(t0[:], ms[0][:], c1[:])
                    nc.vector.tensor_add(
                        zt[:, 2 * vh : 2 * vh + 2], t0[:], ms[2][:]
                    )
                    t1 = zpool.tile([128, 2, 8, 32], BF16, tag="zp", name="t1")
                    nc.vector.tensor_sub(t1[:], c1[:], ms[2][:])
                    nc.vector.tensor_sub(
                        zt[:, 4 + 2 * vh : 4 + 2 * vh + 2], t1[:], ms[3][:]
                    )
                # prefetch next quarter's V one quarter ahead
                if occ == 0 and qi + 1 < len(quarters):
                    vqs[qi + 1] = emit_v_dma(*quarters[qi + 1])
                # Y col stage (Z A), batched over both zu via stride-4 dim1
                # slices: Y[..,zv0] = Zv0+Zv1+Zv2 ; Y[..,zv1] = Zv1-Zv2-Zv3.
                # y layout (zu, ty, (tx, zv)): out col = 2*tx + zv.
                yt = ypool.tile([128, 2, 8, 64], F32, tag="y", name="y")
                ta = zpool.tile([128, 2, 8, 32], BF16, tag="zp", name="ta")
                nc.vector.tensor_add(ta[:], zt[:, 0:8:4], zt[:, 1:8:4])
                nc.vector.tensor_add(yt[:, :, :, 0::2], ta[:], zt[:, 2:8:4])
                tb = zpool.tile([128, 2, 8, 32], BF16, tag="zp", name="tb")
                nc.vector.tensor_sub(tb[:], zt[:, 1:8:4], zt[:, 2:8:4])
                nc.vector.tensor_sub(yt[:, :, :, 1::2], tb[:], zt[:, 3:8:4])
                for zu in range(2):
                    r0 = 16 * q + zu
                    nc.sync.dma_start(
                        o_d[b, occ * 128 : (occ + 1) * 128, r0 : r0 + 15 : 2, :],
                        yt[:, zu],
                    )
                # b1's weights stream in as b0's retire
                if b == 0 and q == 3:
                    wts[(1, occ)] = emit_w_dma(1, occ)

    nc.compile()
    return nc


def get_nc(**kwargs):
    key = tuple(sorted(kwargs.items()))
    if key not in _NC_CACHE:
        _NC_CACHE[key] = _build(**kwargs)
    return _NC_CACHE[key]


def make_in_maps(x, s, weight):
    """Shard full inputs into 8 per-core input maps (host folds the
    modulation and applies both Winograd transforms)."""
    x = np.asarray(x, dtype=np.float32)
    s = np.asarray(s, dtype=np.float32)
    weight = np.asarray(weight, dtype=np.float32)

    wm = weight[None] * (s[:, None, :, None, None] * CSCALE)  # [B,o,i,3,3]
    sig_inv = 1.0 / np.sqrt(
        (wm.astype(np.float64) ** 2).sum(axis=(2, 3, 4), keepdims=True) + EPS
    )
    weff = wm * sig_inv.astype(np.float32)

    G = np.array(
        [[1, 0, 0], [0.5, 0.5, 0.5], [0.5, -0.5, 0.5], [0, 0, 1]], np.float32
    )
    wt = np.einsum("up,soipq,vq->suvoi", G, weff, G, optimize=True)
    # device layout [b, occ, p_i, u*4+v, icc, oc_in]
    w_prep = (
        wt.reshape(B, 4, 4, NCH, 128, NCH, 128)
        .transpose(0, 3, 6, 1, 2, 5, 4)
        .reshape(B, NCH, 128, 16, NCH, 128)
        .astype(BF16_NP)
    )

    # data transform V = B^T d B per sample (f32, one final bf16 round)
    v_prep = np.empty((B, 4, 128, 64, 8, 32), dtype=BF16_NP)
    r2 = np.arange(32) * 2
    c2 = np.arange(32) * 2
    for bi in range(B):
        xp = np.zeros((C, HP, HP), np.float32)
        xp[:, 1 : H + 1, 1 : W + 1] = x[bi]
        u0 = xp[:, r2, :] - xp[:, r2 + 2, :]
        u1 = xp[:, r2 + 1, :] + xp[:, r2 + 2, :]
        u2 = xp[:, r2 + 2, :] - xp[:, r2 + 1, :]
        u3 = xp[:, r2 + 1, :] - xp[:, r2 + 3, :]
        uu = np.stack([u0, u1, u2, u3])  # [4u, 512, 32ty, 66]
        vv = np.stack(
            [
                uu[:, :, :, c2] - uu[:, :, :, c2 + 2],
                uu[:, :, :, c2 + 1] + uu[:, :, :, c2 + 2],
                uu[:, :, :, c2 + 2] - uu[:, :, :, c2 + 1],
                uu[:, :, :, c2 + 1] - uu[:, :, :, c2 + 3],
            ],
            axis=1,
        )  # [4u, 4v, 512, 32ty, 32tx]
        v_prep[bi] = (
            vv.reshape(4, 4, NCH, 128, 4, 8, 32)
            .transpose(4, 3, 0, 1, 2, 5, 6)
            .reshape(4, 128, 64, 8, 32)
            .astype(BF16_NP)
        )

    in_maps = []
    for core in range(N_CORES):
        sl = slice(core * B2, (core + 1) * B2)
        in_maps.append(
            {
                "v": np.ascontiguousarray(v_prep[sl]),
                "w": np.ascontiguousarray(w_prep[sl]),
            }
        )
    return in_maps


def kernel(x, s, weight):
    nc = get_nc()
    in_maps = make_in_maps(x, s, weight)
    res = run_bass_kernel_spmd(nc, in_maps, list(range(N_CORES)))
    out = np.concatenate([r["o"] for r in res.results], axis=0)
    return out.astype(np.float32)


# revision 17
# speedup vs baseline: 1.0020x; 1.0020x over previous
"""ConvModLayer (StyleGAN2-style modulated 3x3 conv) on 8 Trainium2
NeuronCores — data-parallel over the batch (16 samples -> 2 per core),
computed via Winograd F(2x2, 3x3) in bf16.

Host folds modulation/demodulation into per-sample weights, applies the
Winograd filter transform G w G^T, AND the data transform B^T d B (so
the device receives ready-to-matmul V tiles in bf16):
  weff[b,o,i]   = w * c * s[b,i] * rsqrt(sigma_sq[b,o]+eps)
  Wt[b,uv,o,i]  = (G weff G^T)[uv]
  V[b,uv,i,t]   = (B^T d B)[uv]   per 4x4 input tile (stride 2)
Device per quarter-image (8 tile-rows x 32 tile-cols = 256 tiles of
2x2 outputs) and output-channel chunk:
  M[uv] = Wt[uv] @ V[uv]      -- 64 matmuls (free 256), PSUM f32
  Y = A^T M A                 -- batched DVE adds (ACT helps drain PSUM)
16*512*512*1024 MACs per sample vs 9*512*512*4096 direct: 2.25x fewer
PE cycles (2048 matmuls of 256 rows vs 2304 of 512).
"""

import sys
from contextlib import ExitStack

if "/opt/trn_rl_repo" not in sys.path:
    sys.path.insert(0, "/opt/trn_rl_repo")

import ml_dtypes
import numpy as np

import concourse.bacc as bacc
import concourse.mybir as mybir
import concourse.tile as tile
from concourse.bass_utils import run_bass_kernel_spmd

F32 = mybir.dt.float32
BF16 = mybir.dt.bfloat16
BF16_NP = np.dtype(ml_dtypes.bfloat16)

N_CORES = 8
B = 16
B2 = B // N_CORES  # samples per core
C = 512
NCH = 4  # 128-partition channel chunks
H = W = 64
HP = H + 2  # zero-padded
EPS = 1e-8
CSCALE = 1.0 / (C * 9) ** 0.5

_NC_CACHE = {}


def _build():
    nc = bacc.Bacc("TRN2", target_bir_lowering=False, debug=False)

    v_d = nc.dram_tensor("v", [B2, 4, 128, 64, 8, 32], BF16, kind="ExternalInput")
    w_d = nc.dram_tensor("w", [B2, NCH, 128, 16, NCH, 128], BF16, kind="ExternalInput")
    o_d = nc.dram_tensor("o", [B2, C, H, W], F32, kind="ExternalOutput")

    with tile.TileContext(nc) as tc, ExitStack() as ctx:
        vpool = ctx.enter_context(tc.tile_pool(name="vpool", bufs=2))
        wpool = ctx.enter_context(tc.tile_pool(name="wpool", bufs=5))
        zpool = ctx.enter_context(tc.tile_pool(name="zpool", bufs=8))
        ztpool = ctx.enter_context(tc.tile_pool(name="ztpool", bufs=3))
        ypool = ctx.enter_context(tc.tile_pool(name="ypool", bufs=3))
        pspool = ctx.enter_context(
            tc.tile_pool(name="pspool", bufs=8, space="PSUM")
        )

        quarters = [(b, q) for b in range(B2) for q in range(4)]

        def emit_v_dma(b, q):
            vt = vpool.tile([128, 64, 8, 32], BF16, tag="v", name=f"v{b}{q}")
            nc.sync.dma_start(vt[:], v_d[b, q])
            return vt

        def emit_w_dma(b, occ):
            wt = wpool.tile([128, 16, NCH, 128], BF16, tag="w", name=f"w{b}{occ}")
            nc.sync.dma_start(wt[:], w_d[b, occ])
            return wt

        # warm-up, in first-group need order: the whole w(0,0) tile and the
        # vh0 half of V(0,0) gate the first 32 matmuls, so emit w(0,0)
        # first, then V(0,0) split into vh halves (dim1 8-blocks
        # {u*16+vh*8}); all of b0's weights go before quarter 1's V so the
        # occ 2/3 groups aren't stuck behind a 4.2MB transfer.
        wts = {}
        wts[(0, 0)] = emit_w_dma(0, 0)
        v0 = vpool.tile([128, 64, 8, 32], BF16, tag="v", name="v00")
        for vh in range(2):
            for u in range(4):
                j = u * 16 + vh * 8
                nc.sync.dma_start(v0[:, j : j + 8], v_d[0, 0, :, j : j + 8])
        vqs = {0: v0}
        for occ in range(1, NCH):
            wts[(0, occ)] = emit_w_dma(0, occ)
        vqs[1] = emit_v_dma(0, 1)

        for qi, (b, q) in enumerate(quarters):
            vt = vqs[qi]
            for occ in range(NCH):
                wt = wts[(b, occ)]
                # zt dim1 = zu*4 + v
                zt = ztpool.tile([128, 8, 8, 32], BF16, tag="zt", name="zt")
                for vh in range(2):
                    ms = [
                        pspool.tile([128, 2, 8, 32], F32, tag="ps", name=f"m{u}")
                        for u in range(4)
                    ]
                    for ic in range(NCH):
                        for u in range(4):
                            for vi in range(2):
                                uv = u * 4 + 2 * vh + vi
                                nc.tensor.matmul(
                                    ms[u][:, vi],
                                    wt[:, uv, ic, :],
                                    vt[:, uv * 4 + ic],
                                    start=(ic == 0 and vi == 0),
                                    stop=(ic == 3 and vi == 1),
                                    skip_group_check=True,
                                )
                    # Z row stage (A^T M), batched over both v of this half:
                    #   Z0 = M0+M1+M2 ; Z1 = M1-M2-M3
                    # TensorTensor may read only ONE PSUM input, so M1 goes
                    # through an ACT copy; each DVE op reads one PSUM operand.
                    c1 = zpool.tile([128, 2, 8, 32], BF16, tag="zp", name="c1")
                    nc.scalar.copy(c1[:], ms[1][:])
                    t0 = zpool.tile([128, 2, 8, 32], BF16, tag="zp", name="t0")
                    nc.vector.tensor_add(t0[:], ms[0][:], c1[:])
                    nc.vector.tensor_add(
                        zt[:, 2 * vh : 2 * vh + 2], t0[:], ms[2][:]
                    )
                    t1 = zpool.tile([128, 2, 8, 32], BF16, tag="zp", name="t1")
                    nc.vector.tensor_sub(t1[:], c1[:], ms[2][:])
                    nc.vector.tensor_sub(
                        zt[:, 4 + 2 * vh : 4 + 2 * vh + 2], t1[:], ms[3][:]
                    )
                # prefetch next quarter's V one quarter ahead
                if occ == 0 and qi + 1 < len(quarters):
                    vqs[qi + 1] = emit_v_dma(*quarters[qi + 1])
                # Y col stage (Z A), batched over both zu via stride-4 dim1
                # slices: Y[..,zv0] = Zv0+Zv1+Zv2 ; Y[..,zv1] = Zv1-Zv2-Zv3.
                # y layout (zu, ty, (tx, zv)): out col = 2*tx + zv.
                yt = ypool.tile([128, 2, 8, 64], F32, tag="y", name="y")
                ta = zpool.tile([128, 2, 8, 32], BF16, tag="zp", name="ta")
                nc.vector.tensor_add(ta[:], zt[:, 0:8:4], zt[:, 1:8:4])
                nc.vector.tensor_add(yt[:, :, :, 0::2], ta[:], zt[:, 2:8:4])
                tb = zpool.tile([128, 2, 8, 32], BF16, tag="zp", name="tb")
                nc.vector.tensor_sub(tb[:], zt[:, 1:8:4], zt[:, 2:8:4])
                nc.vector.tensor_sub(yt[:, :, :, 1::2], tb[:], zt[:, 3:8:4])
                for zu in range(2):
                    r0 = 16 * q + zu
                    nc.sync.dma_start(
                        o_d[b, occ * 128 : (occ + 1) * 128, r0 : r0 + 15 : 2, :],
                        yt[:, zu],
                    )
                # b1's weights stream in as b0's retire
                if b == 0 and q == 3:
                    wts[(1, occ)] = emit_w_dma(1, occ)

    nc.compile()
    return nc


def get_nc(**kwargs):
    key = tuple(sorted(kwargs.items()))
    if key not in _NC_CACHE:
        _NC_CACHE[key] = _build(**kwargs)
    return _NC_CACHE[key]


def make_in_maps(x, s, weight):
    """Shard full inputs into 8 per-core input maps (host folds the
    modulation and applies both Winograd transforms)."""
    x = np.asarray(x, dtype=np.float32)
    s = np.asarray(s, dtype=np.float32)
    weight = np.asarray(weight, dtype=np.float32)

    wm = weight[None] * (s[:, None, :, None, None] * CSCALE)  # [B,o,i,3,3]
    sig_inv = 1.0 / np.sqrt(
        (wm.astype(np.float64) ** 2).sum(axis=(2, 3, 4), keepdims=True) + EPS
    )
    weff = wm * sig_inv.astype(np.float32)

    G = np.array(
        [[1, 0, 0], [0.5, 0.5, 0.5], [0.5, -0.5, 0.5], [0, 0, 1]], np.float32
    )
    wt = np.einsum("up,soipq,vq->suvoi", G, weff, G, optimize=True)
    # device layout [b, occ, p_i, u*4+v, icc, oc_in]
    w_prep = (
        wt.reshape(B, 4, 4, NCH, 128, NCH, 128)
        .transpose(0, 3, 6, 1, 2, 5, 4)
        .reshape(B, NCH, 128, 16, NCH, 128)
        .astype(BF16_NP)
    )

    # data transform V = B^T d B per sample (f32, one final bf16 round)
    v_prep = np.empty((B, 4, 128, 64, 8, 32), dtype=BF16_NP)
    r2 = np.arange(32) * 2
    c2 = np.arange(32) * 2
    for bi in range(B):
        xp = np.zeros((C, HP, HP), np.float32)
        xp[:, 1 : H + 1, 1 : W + 1] = x[bi]
        u0 = xp[:, r2, :] - xp[:, r2 + 2, :]
        u1 = xp[:, r2 + 1, :] + xp[:, r2 + 2, :]
        u2 = xp[:, r2 + 2, :] - xp[:, r2 + 1, :]
        u3 = xp[:, r2 + 1, :] - xp[:, r2 + 3, :]
        uu = np.stack([u0, u1, u2, u3])  # [4u, 512, 32ty, 66]
        vv = np.stack(
            [
                uu[:, :, :, c2] - uu[:, :, :, c2 + 2],
                uu[:, :, :, c2 + 1] + uu[:, :, :, c2 + 2],
                uu[:, :, :, c2 + 2] - uu[:, :, :, c2 + 1],
                uu[:, :, :, c2 + 1] - uu[:, :, :, c2 + 3],
            ],
            axis=1,
        )  # [4u, 4v, 512, 32ty, 32tx]
        v_prep[bi] = (
            vv.reshape(4, 4, NCH, 128, 4, 8, 32)
            .transpose(4, 3, 0, 1, 2, 5, 6)
            .reshape(4, 128, 64, 8, 32)
            .astype(BF16_NP)
        )

    in_maps = []
    for core in range(N_CORES):
        sl = slice(core * B2, (core + 1) * B2)
        in_maps.append(
            {
                "v": np.ascontiguousarray(v_prep[sl]),
                "w": np.ascontiguousarray(w_prep[sl]),
            }
        )
    return in_maps


def kernel(x, s, weight):
    nc = get_nc()
    in_maps = make_in_maps(x, s, weight)
    res = run_bass_kernel_spmd(nc, in_maps, list(range(N_CORES)))
    out = np.concatenate([r["o"] for r in res.results], axis=0)
    return out.astype(np.float32)
